# revision 53
# baseline (speedup 1.0000x reference)
"""BiLSTM Trainium2 kernel — transposed (weight-stationary) design.

Problem: B=32, T=512, I=512, H=512 bidirectional LSTM (torch gate order
i,f,g,o; shared weights across directions; backward outputs stacked in
processing order).

Sharding: 8 cores = 2 directions x 4 batch groups of 8 rows. Every core runs
the IDENTICAL program; backward cores get time-reversed x (host prep).

Layout (the key idea): hidden dim lives on PARTITIONS, batch (8) is the
matmul moving dim. The 2048-wide gate dim is 16 chunks of 128 partitions,
chunk c -> (gate g=c//4, hidden-chunk q=c%4), gate order [i,f,g,o].

  - gx windows of W=4 steps are precomputed into PSUM bank tiles
    [128, 16 chunks, 32] (one full PSUM bank each), bias folded in via a
    K=1 ones-row matmul. The per-step recurrence h @ Wh.T accumulates
    INTO the same PSUM region (start=False), so there are no PSUM-init
    "selector" matmuls and no gx round trip.
  - Recurrent matmuls are weight-stationary: lhsT = WhT 128x128 chunk,
    rhs = h [128, 8]. 64 matmuls x 8 moving rows per step (bf16).
  - Sigmoid-only epilogue: g-gate weights are pre-scaled x2 on the host so
    tanh(g) = 2*sigmoid(2g)-1; the cell state is kept as c/2 and h as h/2
    (Wh pre-scaled x2 to compensate, y rescaled x2 on the host):
        fc = sigm(f) * c_half
        v  = (sigm(2g) - 0.5) * sigm(i)        [scalar_tensor_tensor]
        c_half' = v + fc
        sc = sigmoid(4 * c_half')               [activation scale=4]
        h_half = (sc - 0.5) * sigm(o)           [scalar_tensor_tensor]
    h_half is written in bf16 directly into the big y SBUF buffer, which
    doubles as the matmul rhs for the next step. One DMA stores y at the end.
"""

import numpy as np
from ml_dtypes import bfloat16

B, T, I, H = 32, 512, 512, 512
G4 = 4 * H          # 2048 gate width
BL = 8              # batch rows per core
W = 4               # steps per PSUM window bank
NCH = 16            # gate-dim chunks of 128
AHEAD = 3           # windows of gx lookahead

_COMPILED = {}


def _build_program(t_steps: int):
    import concourse.bass as bass
    import concourse.tile as tile
    from concourse import bacc, mybir

    dt = mybir.dt
    f32 = dt.float32
    bf16 = dt.bfloat16
    add = mybir.AluOpType.add
    mult = mybir.AluOpType.mult
    sigf = mybir.ActivationFunctionType.Sigmoid

    nw = t_steps // W

    nc = bacc.Bacc("TRN2", target_bir_lowering=False, debug=False)

    xT_d = nc.declare_dram_parameter("xT", [I, t_steps * BL], bf16, isOutput=False)
    whT_d = nc.declare_dram_parameter("whT", [H, G4], bf16, isOutput=False)
    wxT_d = nc.declare_dram_parameter("wxT", [I, G4], bf16, isOutput=False)
    b_d = nc.declare_dram_parameter("b1p", [1, G4], bf16, isOutput=False)
    ones_d = nc.declare_dram_parameter("ones", [1, W * BL], bf16, isOutput=False)
    y_d = nc.declare_dram_parameter("y", [128, t_steps, 4, BL], bf16, isOutput=True)

    with tile.TileContext(nc) as tc:
        with (
            tc.tile_pool(name="const", bufs=1) as cpool,
            tc.tile_pool(name="bank", bufs=4, space="PSUM") as bankpool,
            tc.tile_pool(name="banko", bufs=4, space="PSUM") as bankopool,
            tc.tile_pool(name="ep", bufs=2) as ep,
            tc.tile_pool(name="cst", bufs=2) as cst,
        ):
            # ---- constants ----
            # spread the input DMAs across the three HWDGE-capable queues
            # (SP/ACT/DVE) and order them by first use, so descriptor-gen and
            # transfers overlap instead of serializing ~25us on one queue.
            dmaq = [nc.sync, nc.scalar]
            qi = [0]

            def dma(out, in_):
                dmaq[qi[0] % 2].dma_start(out=out, in_=in_)
                qi[0] += 1

            nxc = 4 if t_steps >= 64 else 1
            xc = t_steps * BL // nxc
            xT = [
                cpool.tile([128, t_steps * BL], bf16, tag=f"xT{k}", name=f"xT{k}")
                for k in range(4)
            ]
            wxT = [
                cpool.tile([128, G4], bf16, tag=f"wxT{k}", name=f"wxT{k}")
                for k in range(4)
            ]
            whT = [
                cpool.tile([128, G4], bf16, tag=f"whT{k}", name=f"whT{k}")
                for k in range(4)
            ]
            b1p = cpool.tile([1, G4], bf16, tag="b1p")
            ones = cpool.tile([1, W * BL], bf16, tag="ones")
            dma(b1p, b_d[:, :])
            dma(ones, ones_d[:, :])
            for k in range(4):
                dma(wxT[k], wxT_d[k * 128 : (k + 1) * 128, :])
                dma(xT[k][:, 0:xc], xT_d[k * 128 : (k + 1) * 128, 0:xc])
            for k in range(4):
                dma(whT[k], whT_d[k * 128 : (k + 1) * 128, :])
            for c in range(1, nxc):
                for k in range(4):
                    dma(
                        xT[k][:, c * xc : (c + 1) * xc],
                        xT_d[k * 128 : (k + 1) * 128, c * xc : (c + 1) * xc],
                    )

            # y buffer: slot 0 is h_{-1} = 0, slot t+1 holds h_half(t) in bf16
            y_sb = cpool.tile([128, t_steps + 1, 4, BL], bf16, tag="y")
            nc.vector.memset(y_sb[:, 0], 0.0)
            c_prev = cst.tile([128, 4, BL], f32, tag="c", name="c_init")
            nc.vector.memset(c_prev, 0.0)
            # dummy sigmoid so the ACT table load runs at t~0, not gated with
            # the first real activation behind the window-0 matmuls
            dummy = cst.tile([1, 1], f32, tag="dummy")
            nc.vector.memset(dummy, 0.0)
            nc.scalar.activation(dummy, dummy, sigf)

            # ---- gx windows ----
            # ifg and o gates live in SEPARATE psum tiles so the o-chunk rec
            # matmuls can be emitted after sigma_ifg without a false WAR;
            # sigma_ifg's gate then covers only the 48 ifg matmuls.
            banks = {}

            def emit_gx(w, cs):
                if w >= nw:
                    return
                if w not in banks:
                    banks[w] = (
                        bankpool.tile(
                            [128, 12, W * BL], f32, tag="bank", name=f"bank{w}"
                        ),
                        bankopool.tile(
                            [128, 4, W * BL], f32, tag="banko", name=f"banko{w}"
                        ),
                    )
                bki, bko = banks[w]
                for c in cs:
                    out = bki[:, c, :] if c < 12 else bko[:, c - 12, :]
                    for k in range(4):
                        # start=True lazily zeroes the whole psum bank:
                        # exactly one per tile (chunk 0 / chunk 12)
                        nc.tensor.matmul(
                            out,
                            lhsT=wxT[k][:, c * 128 : (c + 1) * 128],
                            rhs=xT[k][:, w * W * BL : (w + 1) * W * BL],
                            start=(k == 0 and c in (0, 12)),
                            stop=False,
                            skip_group_check=True,
                        )
                    nc.tensor.matmul(
                        out,
                        lhsT=b1p[0:1, c * 128 : (c + 1) * 128],
                        rhs=ones[0:1, :],
                        start=False,
                        stop=False,
                        skip_group_check=True,
                    )

            emit_gx(0, range(NCH))

            # ---- main loop ----
            for t in range(t_steps):
                w, j = divmod(t, W)
                bki, bko = banks[w]
                jsl = slice(j * BL, (j + 1) * BL)

                def rec_mms(tile, cs, coff):
                    if t == 0:  # h(-1)=0: no recurrent contribution
                        return
                    for c in cs:
                        for k in range(4):
                            nc.tensor.matmul(
                                tile[:, c - coff, jsl],
                                lhsT=whT[k][:, c * 128 : (c + 1) * 128],
                                rhs=y_sb[:, t, k, :],
                                start=False,
                                stop=(k == 3),
                                skip_group_check=True,
                            )

                rec_mms(bki, range(12), 0)
                s_ifg = ep.tile([128, 12, BL], f32, tag="sifg", name=f"sifg{t}")
                nc.scalar.activation(s_ifg, bki[:, :, jsl], sigf)
                rec_mms(bko, range(12, 16), 12)
                so = ep.tile([128, 4, BL], bf16, tag="so", name=f"so{t}")
                nc.scalar.activation(so, bko[:, :, jsl], sigf)

                v = ep.tile([128, 4, BL], f32, tag="v", name=f"v{t}")
                nc.vector.scalar_tensor_tensor(
                    v, s_ifg[:, 8:12, :], -0.5, s_ifg[:, 0:4, :], add, mult
                )
                fc = ep.tile([128, 4, BL], f32, tag="fc", name=f"fc{t}")
                nc.vector.tensor_mul(fc, s_ifg[:, 4:8, :], c_prev)
                c_new = cst.tile([128, 4, BL], f32, tag="c", name=f"c{t}")
                nc.vector.tensor_add(c_new, v, fc)
                sc = ep.tile([128, 4, BL], bf16, tag="sc", name=f"sc{t}")
                nc.scalar.activation(sc, c_new, sigf, scale=4.0)
                nc.vector.scalar_tensor_tensor(
                    y_sb[:, t + 1], sc, -0.5, so, add, mult
                )
                c_prev = c_new
                # future gx emitted at the BOTTOM of the body so no sigma
                # gate ever counts it; windows 1..AHEAD spread over steps 0..3
                if t < W:
                    for wx in range(1, AHEAD + 1):
                        emit_gx(wx, range(4 * j, 4 * j + 4))
                else:
                    emit_gx(w + AHEAD, range(4 * j, 4 * j + 4))
                if j == W - 1:
                    del banks[w]
                nyc = t_steps // 64
                if t_steps >= 64 and (t + 1) % nyc == 0:
                    chk = (t + 1) // nyc - 1
                    nc.sync.dma_start(
                        out=y_d[:, chk * nyc : (chk + 1) * nyc],
                        in_=y_sb[:, 1 + chk * nyc : 1 + (chk + 1) * nyc],
                    )
            if t_steps < 64:
                nc.sync.dma_start(out=y_d[:, :, :, :], in_=y_sb[:, 1:])

    nc.compile()
    return nc


def _get_program(t_steps: int):
    if t_steps not in _COMPILED:
        _COMPILED[t_steps] = _build_program(t_steps)
    return _COMPILED[t_steps]


def _host_prep(x, Wx, bx, Wh, bh, t_steps):
    # gate order already [i,f,g,o] (torch). Scales: h is carried as h/2 so
    # Wh cols x2; g rows additionally x2 (tanh via sigmoid) -> Wh g-cols x4,
    # Wx g-cols x2, bias g x2.
    wh_scale = np.full(G4, 2.0, np.float32)
    wh_scale[2 * H : 3 * H] = 4.0
    wx_scale = np.ones(G4, np.float32)
    wx_scale[2 * H : 3 * H] = 2.0
    whT = np.ascontiguousarray((Wh.T * wh_scale[None, :]).astype(bfloat16))
    wxT = np.ascontiguousarray((Wx.T * wx_scale[None, :]).astype(bfloat16))
    b = ((bx + bh) * wx_scale).astype(bfloat16).reshape(1, G4)
    ones = np.ones((1, W * BL), bfloat16)
    in_maps = []
    for core in range(8):
        d, g = divmod(core, 4)
        xc = x[g * BL : (g + 1) * BL, :t_steps]  # [8, T, I]
        if d == 1:
            xc = xc[:, ::-1]
        xTc = np.ascontiguousarray(
            xc.transpose(2, 1, 0).reshape(I, t_steps * BL).astype(bfloat16)
        )
        in_maps.append(
            {"xT": xTc, "whT": whT, "wxT": wxT, "b1p": b, "ones": ones}
        )
    return in_maps


def kernel(x, Wx, bx, Wh, bh):
    from concourse.bass_utils import run_bass_kernel_spmd

    x = np.asarray(x, dtype=np.float32)
    Wx = np.asarray(Wx, dtype=np.float32)
    bx = np.asarray(bx, dtype=np.float32)
    Wh = np.asarray(Wh, dtype=np.float32)
    bh = np.asarray(bh, dtype=np.float32)
    nc = _get_program(T)
    in_maps = _host_prep(x, Wx, bx, Wh, bh, T)
    res = run_bass_kernel_spmd(nc, in_maps, list(range(8)))
    out = np.empty((B, T, 2 * H), dtype=np.float32)
    for core in range(8):
        d, g = divmod(core, 4)
        y = np.asarray(res.results[core]["y"], dtype=np.float32)  # [128,T,4,8]
        # h[b, t, q*128+p] = 2 * y[p, t, q, b]
        yh = 2.0 * y.transpose(3, 1, 2, 0).reshape(BL, T, H)
        out[g * BL : (g + 1) * BL, :, d * H : (d + 1) * H] = yh
    return out


def _np_lstm(x, Wx, bx, Wh, bh):
    """Single-direction numpy reference for self-test (forward order)."""
    b_, t_, _ = x.shape
    h = np.zeros((b_, H), np.float32)
    c = np.zeros((b_, H), np.float32)
    gx = x @ Wx.T + bx
    ys = []
    for t in range(t_):
        gates = gx[:, t] + h @ Wh.T + bh
        i_g, f_g, g_g, o_g = np.split(gates, 4, axis=1)
        i_t = 1 / (1 + np.exp(-i_g))
        f_t = 1 / (1 + np.exp(-f_g))
        g_t = np.tanh(g_g)
        o_t = 1 / (1 + np.exp(-o_g))
        c = c * f_t + i_t * g_t
        h = o_t * np.tanh(c)
        ys.append(h)
    return np.stack(ys, 1)


def _selftest(t_steps=16):
    from concourse.bass_interp import CoreSim

    rng = np.random.default_rng(0)
    s = 1.0 / np.sqrt(H)
    x = rng.standard_normal((B, T, I), dtype=np.float32)
    Wx = (rng.standard_normal((G4, I), dtype=np.float32) * s).astype(np.float32)
    bx = (rng.standard_normal(G4) * s).astype(np.float32)
    Wh = (rng.standard_normal((G4, H), dtype=np.float32) * s).astype(np.float32)
    bh = (rng.standard_normal(G4) * s).astype(np.float32)

    nc = _get_program(t_steps)
    in_maps = _host_prep(x, Wx, bx, Wh, bh, t_steps)
    sim = CoreSim(nc, trace=False)
    for k, v in in_maps[0].items():
        sim.tensor(k)[:] = v
    sim.simulate()
    y = np.asarray(sim.tensor("y"), dtype=np.float32)  # [128, t, 4, 8]
    yh = 2.0 * y.transpose(3, 1, 2, 0).reshape(BL, t_steps, H)
    ref = _np_lstm(x[:BL, :t_steps], Wx, bx, Wh, bh)  # [BL, t, H]
    err = np.abs(yh - ref)
    scale = np.abs(ref).max()
    print(f"selftest T={t_steps}: max abs err {err.max():.3e} (scale {scale:.3f})")
    return err.max()


if __name__ == "__main__":
    _selftest(16)
